# revision 1
# baseline (speedup 1.0000x reference)
"""Trainium2 Bass kernel for nn_AttModel (masked-attention GNN message passing).

Computation (per batch b):
    q/k/v = relu(x @ W*^T + b*)            [N, H]
    S     = q @ k^T                        [N, N]
    att   = softmax(S*mask - NEG*(1-mask)) [N, N]
    y     = relu((att @ v) @ Wo^T + bo)    [N, DOUT]

Sharding: data-parallel over (batch, query-half) -> 8 cores, each owning
2048 query rows of one batch. Zero cross-core communication; each core
streams its mask slice once (the memory roofline).

Per-core algorithm (mode "stht", the default):
    - As part of host-side sharding, each core's mask slice is laid out
      transposed ([key, query]) and encoded bf16 (exact for 0/1 values),
      so it streams straight into the masking op with no on-chip
      transposes, casts, or copies.
    - Scores need no max-subtraction: q,k >= 0 post-relu and
      max(S) ~ 33 << 88, so exp(S) is safe in f32.
    - Scores are computed directly transposed: S^T[j, i] tiles with keys
      on partitions (lhsT = kT chunk, rhs = qT group). One fused DVE op
      applies the mask: L = (S^T + C) * maskT, written to SBUF so the
      PSUM slot recycles early; ACT computes P^T = exp(L - C) straight
      into SBUF bf16 (mask=0 -> exp(-C) == 0 exactly, C=500), one
      [128, 2048] instruction per two score tiles.
    - att @ v contracts keys-on-partitions with P^T as the moving
      operand; the softmax denominator Z (column sums of P^T) comes from
      interleaved rank-1 ones-matmuls on PE.
    - The whole 64-step stream is software-pipelined (S-matmuls run two
      steps ahead of the mask/exp chain; 3 PSUM score slots).
    - Epilogue per 512-query group: Z moves onto the query-partition
      axis via four k=1 transpose matmuls, then
      y = relu(G/Z + bo) via one fused DVE op + one ACT relu.

Environment note: this walrus build rejects instructions with more than
one semaphore wait; _split_multiwaits rewrites the Tile output to
single-wait form (extra waits move to preceding NoOps, same semantics).
"""
import ml_dtypes
import numpy as np

import concourse.bass as bass
import concourse.mybir as mybir
from concourse.tile import TileContext
from concourse.bass_utils import run_bass_kernel_spmd
from concourse.masks import make_identity
from concourse import bass_isa

B, N, DIN, H, DOUT = 4, 4096, 128, 128, 128
NQ = N // 2  # 2048 query rows per core
CORES = 8
C_SHIFT = 500.0

F32 = mybir.dt.float32
BF16 = mybir.dt.bfloat16
AL = mybir.AluOpType
AF = mybir.ActivationFunctionType

_wfix_ctr = [0]


def _split_multiwaits(nc):
    """This walrus build rejects instructions carrying >1 semaphore wait.

    Move all but the last wait of each instruction onto preceding NoOps on
    the same engine (engine streams execute in order, so semantics hold).
    """
    for func in nc.m.functions:
        for block in func.blocks:
            new_insts = []
            changed = False
            for inst in block.instructions:
                si = inst.sync_info
                waits = list(si.on_wait) if si is not None else []
                if len(waits) > 1:
                    for w in waits[:-1]:
                        _wfix_ctr[0] += 1
                        nop = mybir.InstNoOp(
                            name=f"WFIX-{_wfix_ctr[0]}", ins=[], outs=[]
                        )
                        nop.engine = inst.engine
                        nop.sync_info = mybir.SyncInfo(on_wait=[w], on_update=[])
                        new_insts.append(nop)
                    si.on_wait = [waits[-1]]
                    inst.sync_info = si
                    changed = True
                new_insts.append(inst)
            if changed:
                block.instructions = new_insts


def _build_nc(mode="stht", z_gps_mod=1, mask_bf16=True):
    nc = bass.Bass()

    x_kv = nc.dram_tensor("x_kv", [N, DIN], F32, kind="ExternalInput")
    x_q = nc.dram_tensor("x_q", [NQ, DIN], F32, kind="ExternalInput")
    if mode == "stht":
        mask_d = nc.dram_tensor(
            "mask", [N, NQ], BF16 if mask_bf16 else F32, kind="ExternalInput"
        )
    else:
        mask_d = nc.dram_tensor("mask", [NQ, N], F32, kind="ExternalInput")
    wq_d = nc.dram_tensor("Wq", [H, DIN], F32, kind="ExternalInput")
    wk_d = nc.dram_tensor("Wk", [H, DIN], F32, kind="ExternalInput")
    wv_d = nc.dram_tensor("Wv", [H, DIN], F32, kind="ExternalInput")
    wo_d = nc.dram_tensor("Wo", [DOUT, H], F32, kind="ExternalInput")
    bq_d = nc.dram_tensor("bq", [H], F32, kind="ExternalInput")
    bk_d = nc.dram_tensor("bk", [H], F32, kind="ExternalInput")
    bv_d = nc.dram_tensor("bv", [H], F32, kind="ExternalInput")
    bo_d = nc.dram_tensor("bo", [DOUT], F32, kind="ExternalInput")
    y_d = nc.dram_tensor("y", [NQ, DOUT], F32, kind="ExternalOutput")

    with TileContext(nc) as tc:
        with tc.tile_pool(name="singles", bufs=1) as singles:
            ident = singles.tile([128, 128], F32, tag="ident")
            make_identity(nc, ident)
            ident16 = singles.tile([128, 128], BF16, tag="ident16")
            nc.vector.tensor_copy(ident16, ident)
            negc = singles.tile([128, 1], F32, tag="negc")
            nc.vector.memset(negc, -C_SHIFT)
            ones16 = singles.tile([128, 1], BF16, tag="ones16")
            nc.vector.memset(ones16, 1.0)
            one_f32 = singles.tile([1, 1], F32, tag="one_f32")
            nc.vector.memset(one_f32, 1.0)

            # per-partition biases for the q/k/v relu epilogue
            bq_sb = singles.tile([128, 1], F32, tag="bq")
            bk_sb = singles.tile([128, 1], F32, tag="bk")
            bv_sb = singles.tile([128, 1], F32, tag="bv")
            bo_sb = singles.tile([128, 128], F32, tag="bo")
            nc.gpsimd.dma_start(
                out=bq_sb, in_=bq_d[:].rearrange("(p o) -> p o", o=1)
            )
            nc.gpsimd.dma_start(
                out=bk_sb, in_=bk_d[:].rearrange("(p o) -> p o", o=1)
            )
            nc.gpsimd.dma_start(
                out=bv_sb, in_=bv_d[:].rearrange("(p o) -> p o", o=1)
            )
            nc.gpsimd.dma_start(
                out=bo_sb,
                in_=bass.AP(tensor=bo_d, offset=0, ap=[[0, 128], [1, 128]]),
            )

            wqT = singles.tile([128, 128], BF16, tag="wqT")
            wkT = singles.tile([128, 128], BF16, tag="wkT")
            wvT = singles.tile([128, 128], BF16, tag="wvT")
            woT = singles.tile([128, 128], F32, tag="woT")
            qT = [
                singles.tile([128, 4, 128], BF16, tag=f"qT{g}", name=f"qT{g}")
                for g in range(4)
            ]
            kT = [
                singles.tile([128, 4, 128], BF16, tag=f"kT{g}", name=f"kT{g}")
                for g in range(8)
            ]
            vn = [
                singles.tile([128, 4, 128], BF16, tag=f"vn{g}", name=f"vn{g}")
                for g in range(8)
            ]

            with (
                tc.tile_pool(name="setup_sb", bufs=3) as ssb,
                tc.tile_pool(name="setup_one", bufs=1) as sone,
                tc.tile_pool(name="setup_ps", bufs=2, space="PSUM") as sps,
            ):
                xT_kv = sone.tile([128, 32, 128], BF16, tag="xTkv")
                xT_q = sone.tile([128, 16, 128], BF16, tag="xTq")
                vT = sone.tile([128, 32, 128], BF16, tag="vT")
                for w_dram, dst in (
                    (wq_d, wqT),
                    (wk_d, wkT),
                    (wv_d, wvT),
                    (wo_d, woT),
                ):
                    wld = ssb.tile([128, 128], F32, tag="wld")
                    nc.sync.dma_start(out=wld, in_=w_dram[:])
                    wps = sps.tile([128, 128], F32, tag="wt")
                    nc.tensor.transpose(wps, wld, ident)
                    nc.scalar.copy(dst, wps)

                xload_kv = ssb.tile([128, 32, 128], F32, tag="xloadkv")
                xv = x_kv[:].rearrange("(c p) d -> p c d", p=128)
                for g4 in range(4):
                    nc.sync.dma_start(
                        out=xload_kv[:, g4 * 8 : (g4 + 1) * 8, :],
                        in_=xv[:, g4 * 8 : (g4 + 1) * 8, :],
                    )
                for g in range(8):
                    xps = sps.tile([128, 4, 128], F32, tag="xt")
                    for k in range(4):
                        nc.tensor.transpose(
                            xps[:, k, :], xload_kv[:, g * 4 + k, :], ident
                        )
                    if g % 2:
                        nc.scalar.copy(xT_kv[:, g * 4 : g * 4 + 4, :], xps)
                    else:
                        nc.vector.tensor_copy(xT_kv[:, g * 4 : g * 4 + 4, :], xps)

                xload_q = ssb.tile([128, 16, 128], F32, tag="xloadq")
                xq = x_q[:].rearrange("(c p) d -> p c d", p=128)
                for g4 in range(2):
                    nc.sync.dma_start(
                        out=xload_q[:, g4 * 8 : (g4 + 1) * 8, :],
                        in_=xq[:, g4 * 8 : (g4 + 1) * 8, :],
                    )
                for g in range(4):
                    xps = sps.tile([128, 4, 128], F32, tag="xt")
                    for k in range(4):
                        nc.tensor.transpose(
                            xps[:, k, :], xload_q[:, g * 4 + k, :], ident
                        )
                    if g % 2:
                        nc.scalar.copy(xT_q[:, g * 4 : g * 4 + 4, :], xps)
                    else:
                        nc.vector.tensor_copy(xT_q[:, g * 4 : g * 4 + 4, :], xps)

                # q/k/v projections interleaved per group so the first
                # S-matmuls (need qT[0], kT[0]) and first att@v matmuls
                # (need vn[0]) all unblock as early as possible.
                for g in range(8):
                    if g < 4:
                        pps = sps.tile([128, 512], F32, tag="qkv",
                                       name=f"pq{g}")
                        nc.tensor.matmul(
                            out=pps,
                            lhsT=wqT,
                            rhs=xT_q[:, g * 4 : g * 4 + 4, :],
                            start=True,
                            stop=True,
                        )
                        nc.scalar.activation(
                            out=qT[g][:], in_=pps, func=AF.Relu, bias=bq_sb
                        )
                    pps = sps.tile([128, 512], F32, tag="qkv", name=f"pk{g}")
                    nc.tensor.matmul(
                        out=pps,
                        lhsT=wkT,
                        rhs=xT_kv[:, g * 4 : g * 4 + 4, :],
                        start=True,
                        stop=True,
                    )
                    nc.scalar.activation(
                        out=kT[g][:], in_=pps, func=AF.Relu, bias=bk_sb
                    )
                    pps = sps.tile([128, 512], F32, tag="qkv", name=f"pv{g}")
                    nc.tensor.matmul(
                        out=pps,
                        lhsT=wvT,
                        rhs=xT_kv[:, g * 4 : g * 4 + 4, :],
                        start=True,
                        stop=True,
                    )
                    nc.scalar.activation(
                        out=vT[:, g * 4 : g * 4 + 4, :],
                        in_=pps,
                        func=AF.Relu,
                        bias=bv_sb,
                    )
                    # V natural [key, h] chunk via PE transpose
                    vps = sps.tile([128, 4, 128], BF16, tag="vt",
                                   name=f"vps{g}")
                    for k in range(4):
                        nc.tensor.transpose(
                            vps[:, k, :], vT[:, g * 4 + k, :], ident16
                        )
                    if g % 2:
                        nc.scalar.copy(vn[g][:], vps)
                    else:
                        nc.vector.tensor_copy(vn[g][:], vps)

            if mode == "stht":
                _main_stht(nc, tc, mask_d, y_d, qT, kT, vn, woT,
                           bo_sb, negc, ones16, one_f32, z_gps_mod,
                           mask_bf16)
            else:
                _main_nat(nc, tc, mask_d, y_d, qT, kT, vn, woT, bo_sb, negc,
                          ident16, use_xbar=(mode == "nat_xbar"))

    _split_multiwaits(nc)
    return nc


def _main_stht(nc, tc, mask_d, y_d, qT, kT, vn, woT, bo_sb, negc,
               ones16, one_f32, z_gps_mod, mask_bf16=True):
    """S^T-orientation main loop with a host-pre-transposed mask
    ([key, query] in DRAM, bf16): the mask streams straight into the
    fused DVE masking op — no cast, no on-chip transposes, no copies.

    Software-pipelined over all 64 (group, jc-pair) steps: S-matmuls run
    LOOKAHEAD steps ahead of the STT/exp/AV chain; the STT writes L to
    SBUF so PSUM slots free early; exp runs as one [128, 2048] ACT
    instruction per two steps to amortize ACT instruction overhead.
    """
    mdt = BF16 if mask_bf16 else F32
    with (
        tc.tile_pool(name="mtp", bufs=6) as mtpool,
        tc.tile_pool(name="ptp", bufs=6) as ptpool,
        tc.tile_pool(name="lbp", bufs=3) as lbpool,
        tc.tile_pool(name="tiny", bufs=8) as tinypool,
        tc.tile_pool(name="utsb", bufs=4) as utsbp,
        tc.tile_pool(name="outb", bufs=8) as ypool,
    ):
      uts_l, zrow_l = [], []
      ig_state = {}
      mq_state = {}
      lb_state = {}
      sps_q = {}
      with (
        tc.tile_pool(name="sps2", bufs=3, space="PSUM") as spsum,
        tc.tile_pool(name="utps", bufs=1, space="PSUM") as utpsum,
        tc.tile_pool(name="zps", bufs=1, space="PSUM") as zpsum,
      ):
        LOOKAHEAD = 4

        def do_av(ppjp, ptp2, woff):
            pig, pjp = divmod(ppjp, 16)
            utp, zp = ig_state[pig]
            for w in range(2):
                jc = 2 * pjp + w
                rhs = ptp2[:, woff + w, :]
                nc.tensor.matmul(
                    out=utp,
                    lhsT=vn[jc // 4][:, jc % 4, :],
                    rhs=rhs,
                    start=(jc == 0),
                    stop=(jc == 31),
                )
                nc.tensor.matmul(
                    out=zp,
                    lhsT=ones16,
                    rhs=rhs,
                    start=(jc == 0),
                    stop=(jc == 31),
                )
            if pjp == 15:
                # group wrap-up: Z row + U^T to SBUF
                zrow = tinypool.tile([1, 512], F32, tag="zrow")
                nc.vector.tensor_copy(zrow, zp)
                uts = utsbp.tile([128, 512], F32, tag="uts")
                nc.scalar.copy(uts, utp)
                uts_l.append(uts)
                zrow_l.append(zrow)

        for step in range(64 + LOOKAHEAD):
            if step < 64:
                ig, jp = divmod(step, 16)
                if jp == 0:
                    utp = utpsum.tile([128, 512], F32, tag="ut",
                                      name=f"utp{ig}")
                    zp = zpsum.tile([1, 512], F32, tag="z", name=f"zp{ig}")
                    ig_state[ig] = (utp, zp)
                if jp % 2 == 0:
                    mquad = mtpool.tile([128, 4, 512], mdt, tag="mt",
                                        name=f"mq{step}")
                    nc.sync.dma_start(
                        out=mquad,
                        in_=mask_d[
                            jp * 256 : (jp + 2) * 256,
                            ig * 512 : (ig + 1) * 512,
                        ].rearrange("(c p) i -> p c i", p=128),
                    )
                    mq_state[ig] = mquad
                mpair = mq_state[ig][:, (jp % 2) * 2 : (jp % 2) * 2 + 2, :]
                sp = spsum.tile([128, 2, 512], F32, tag="s", name=f"sp{step}")
                for w in range(2):
                    jc = 2 * jp + w
                    nc.tensor.matmul(
                        out=sp[:, w, :],
                        lhsT=kT[jc // 4][:, jc % 4, :],
                        rhs=qT[ig][:],
                        start=True,
                        stop=True,
                    )
                sps_q[step] = (sp, mpair)
            if step < LOOKAHEAD:
                continue
            pstep = step - LOOKAHEAD
            sp, mpair = sps_q.pop(pstep)
            spf = sp.rearrange("p a b -> p (a b)")
            # L = (S^T + C) * maskT (one fused DVE op), to SBUF so the
            # PSUM slot frees as soon as the STT has read it.
            lb1 = lbpool.tile([128, 1024], F32, tag="lb", name=f"lb{pstep}")
            nc.vector.scalar_tensor_tensor(
                out=lb1,
                in0=spf,
                scalar=C_SHIFT,
                in1=mpair[:].rearrange("p a b -> p (a b)"),
                op0=AL.add,
                op1=AL.mult,
            )
            ptp2 = ptpool.tile([128, 2, 512], BF16, tag="ptp",
                               name=f"ptp{pstep}")
            nc.scalar.activation(
                out=ptp2[:].rearrange("p a b -> p (a b)"),
                in_=lb1,
                func=AF.Exp,
                bias=negc,
            )
            do_av(pstep, ptp2, 0)
      # epilogue: output projection + normalization for all groups
      with tc.tile_pool(name="epips", bufs=3, space="PSUM") as epipsum:
        for ig in range(4):
            uts, zrow = uts_l[ig], zrow_l[ig]
            # Move Z onto the query-partition axis with four k=1 transpose
            # matmuls, then y = relu(G/Z + bo) via one fused DVE op + relu.
            ztp = epipsum.tile([128, 4], F32, tag="ztp", name=f"ztp{ig}")
            for ib4 in range(4):
                nc.tensor.matmul(
                    out=ztp[:, ib4 : ib4 + 1],
                    lhsT=zrow[:, ib4 * 128 : (ib4 + 1) * 128],
                    rhs=one_f32,
                    start=True,
                    stop=True,
                )
            rz4 = tinypool.tile([128, 4], F32, tag="rz4", name=f"rz4_{ig}")
            nc.vector.reciprocal(rz4, ztp)
            yt4 = ypool.tile([128, 4, 128], F32, tag="y", name=f"y{ig}")
            for ib4 in range(4):
                g = epipsum.tile([128, 128], F32, tag="g",
                                 name=f"g{ig}_{ib4}")
                nc.tensor.matmul(
                    out=g,
                    lhsT=uts[:, ib4 * 128 : (ib4 + 1) * 128],
                    rhs=woT,
                    start=True,
                    stop=True,
                )
                u = ypool.tile([128, 128], F32, tag="u", name=f"u{ig}_{ib4}")
                nc.vector.scalar_tensor_tensor(
                    out=u,
                    in0=g,
                    scalar=rz4[:, ib4 : ib4 + 1],
                    op0=AL.mult,
                    in1=bo_sb,
                    op1=AL.add,
                )
                nc.scalar.activation(out=yt4[:, ib4, :], in_=u, func=AF.Relu)
            nc.sync.dma_start(
                out=y_d[ig * 512 : (ig + 1) * 512, :].rearrange(
                    "(c p) o -> p c o", p=128
                ),
                in_=yt4,
            )


def _main_nat(nc, tc, mask_d, y_d, qT, kT, vn, woT, bo_sb, negc, ident16,
              use_xbar):
    """Natural-orientation fallback: queries on partitions, P transposed
    per 128-row block (XBAR batched DMA, or PE transpose + copies)."""
    with (
        tc.tile_pool(name="maskp", bufs=3) as mpool,
        tc.tile_pool(name="pbuf", bufs=2) as ppool,
        tc.tile_pool(name="ptbuf", bufs=3) as ptpool,
        tc.tile_pool(name="smalls", bufs=8) as smpool,
        tc.tile_pool(name="utsb", bufs=2) as utsbp,
        tc.tile_pool(name="outb", bufs=8) as ypool,
        tc.tile_pool(name="sps2", bufs=2, space="PSUM") as spsum,
        tc.tile_pool(
            name="utps", bufs=2 if use_xbar else 1, space="PSUM"
        ) as utpsum,
        tc.tile_pool(
            name="gps", bufs=2 if use_xbar else 1, space="PSUM"
        ) as gpsum,
        tc.tile_pool(
            name="ptps", bufs=1 if use_xbar else 2, space="PSUM"
        ) as ptpsum,
    ):
        for ib in range(16):
            mt = mpool.tile([128, 4096], F32, tag="mask")
            nc.sync.dma_start(
                out=mt, in_=mask_d[ib * 128 : (ib + 1) * 128, :]
            )
            pb = ppool.tile([128, 4096], BF16, tag="p")
            zp = smpool.tile([128, 4], F32, tag="zp")
            for j2 in range(4):
                sp = spsum.tile([128, 2, 512], F32, tag="s")
                for hh in range(2):
                    s8 = (j2 * 2 + hh) * 4
                    nc.tensor.matmul(
                        out=sp[:, hh, :],
                        lhsT=qT[ib // 4][:, ib % 4, :],
                        rhs=kT[j2 * 2 + hh][:],
                        start=True,
                        stop=True,
                    )
                spf = sp.rearrange("p a b -> p (a b)")
                nc.vector.scalar_tensor_tensor(
                    out=spf,
                    in0=spf,
                    scalar=C_SHIFT,
                    in1=mt[:, j2 * 1024 : (j2 + 1) * 1024],
                    op0=AL.add,
                    op1=AL.mult,
                )
                nc.scalar.activation(
                    out=pb[:, j2 * 1024 : (j2 + 1) * 1024],
                    in_=spf,
                    func=AF.Exp,
                    bias=negc,
                    accum_out=zp[:, j2 : j2 + 1],
                )
            z = smpool.tile([128, 1], F32, tag="z")
            rz = smpool.tile([128, 1], F32, tag="rz")
            nc.vector.tensor_reduce(
                out=z, in_=zp, axis=mybir.AxisListType.X, op=AL.add
            )
            nc.vector.reciprocal(rz, z)
            pt = ptpool.tile([128, 32, 128], BF16, tag="pt")
            if use_xbar:
                nc.sync.dma_start(out=pt, in_=pb, transpose=True)
            else:
                for jc4 in range(8):
                    tps = ptpsum.tile([128, 4, 128], BF16, tag="tp")
                    for k in range(4):
                        jc = jc4 * 4 + k
                        nc.tensor.transpose(
                            tps[:, k, :],
                            pb[:, jc * 128 : (jc + 1) * 128],
                            ident16,
                        )
                    if jc4 % 2:
                        nc.scalar.copy(pt[:, jc4 * 4 : jc4 * 4 + 4, :], tps)
                    else:
                        nc.vector.tensor_copy(
                            pt[:, jc4 * 4 : jc4 * 4 + 4, :], tps
                        )
            utp = utpsum.tile([128, 128], F32, tag="ut")
            for jc in range(32):
                nc.tensor.matmul(
                    out=utp,
                    lhsT=vn[jc // 4][:, jc % 4, :],
                    rhs=pt[:, jc, :],
                    start=(jc == 0),
                    stop=(jc == 31),
                )
            uts = utsbp.tile([128, 128], F32, tag="uts")
            nc.scalar.copy(uts, utp)
            g = gpsum.tile([128, 128], F32, tag="g")
            nc.tensor.matmul(out=g, lhsT=uts, rhs=woT, start=True, stop=True)
            u = ypool.tile([128, 128], F32, tag="u")
            nc.vector.scalar_tensor_tensor(
                out=u,
                in0=bo_sb,
                scalar=z,
                op0=AL.mult,
                in1=g,
                op1=AL.add,
            )
            yt = ypool.tile([128, 128], F32, tag="y")
            nc.scalar.activation(out=yt, in_=u, func=AF.Relu, scale=rz)
            nc.sync.dma_start(out=y_d[ib * 128 : (ib + 1) * 128, :], in_=yt)


_NC_CACHE = {}


def _get_nc(mode="stht", z_gps_mod=1, mask_bf16=True):
    key = (mode, z_gps_mod, mask_bf16)
    if key not in _NC_CACHE:
        _NC_CACHE[key] = _build_nc(mode, z_gps_mod, mask_bf16)
    return _NC_CACHE[key]


def kernel(x, mask, Wv, bv, Wk, bk, Wq, bq, Wo, bo, _trace=False, _mode="stht",
           _z_gps_mod=1, _mask_bf16=True):
    x = np.ascontiguousarray(np.asarray(x, dtype=np.float32))
    mask = np.ascontiguousarray(np.asarray(mask, dtype=np.float32))
    weights = {
        "Wq": np.ascontiguousarray(np.asarray(Wq, np.float32)),
        "Wk": np.ascontiguousarray(np.asarray(Wk, np.float32)),
        "Wv": np.ascontiguousarray(np.asarray(Wv, np.float32)),
        "Wo": np.ascontiguousarray(np.asarray(Wo, np.float32)),
        "bq": np.ascontiguousarray(np.asarray(bq, np.float32)),
        "bk": np.ascontiguousarray(np.asarray(bk, np.float32)),
        "bv": np.ascontiguousarray(np.asarray(bv, np.float32)),
        "bo": np.ascontiguousarray(np.asarray(bo, np.float32)),
    }

    nc = _get_nc(_mode, _z_gps_mod, _mask_bf16)
    mask_dt = ml_dtypes.bfloat16 if (_mode == "stht" and _mask_bf16) else np.float32
    in_maps = []
    for c in range(CORES):
        b, half = divmod(c, 2)
        in_maps.append(
            dict(
                weights,
                x_kv=x[b],
                x_q=np.ascontiguousarray(x[b, half * NQ : (half + 1) * NQ]),
                mask=np.ascontiguousarray(
                    mask[b, half * NQ : (half + 1) * NQ].T.astype(mask_dt)
                    if _mode == "stht"
                    else mask[b, half * NQ : (half + 1) * NQ]
                ),
            )
        )
    res = run_bass_kernel_spmd(
        nc, in_maps, core_ids=list(range(CORES)), trace=_trace
    )
    out = np.empty((B, N, DOUT), dtype=np.float32)
    for c in range(CORES):
        b, half = divmod(c, 2)
        out[b, half * NQ : (half + 1) * NQ] = res.results[c]["y"]
    if _trace:
        return out, res
    return out



# revision 13
# speedup vs baseline: 1.0404x; 1.0404x over previous
"""Trainium2 Bass kernel for nn_AttModel (masked-attention GNN message passing).

Computation (per batch b):
    q/k/v = relu(x @ W*^T + b*)            [N, H]
    S     = q @ k^T                        [N, N]
    att   = softmax(S*mask - NEG*(1-mask)) [N, N]
    y     = relu((att @ v) @ Wo^T + bo)    [N, DOUT]

Sharding: data-parallel over (batch, query-half) -> 8 cores, each owning
2048 query rows of one batch. Zero cross-core communication.

Key identity: exp(S*m - NEG*(1-m)) == m * exp(S) elementwise for m in {0,1}
(exp(S) <= e^33 is finite in bf16/f32), so the kernel computes
    P^T = maskT (*) exp(S^T)
with exp reading the f32 PSUM scores directly on ACT (the critical engine:
64 x [128,1024] exp instructions ~= 66us) and the mask applied afterwards
as an all-bf16 SBUF multiply on DVE at 2x throughput. The softmax
denominator Z is built from two DVE bf16 partial-sum adds (4 key-chunks ->
1) followed by one cheap rank-1 PE ones-matmul per macro-step.

Layout-only host prep (same class as the baseline's mask transpose): x, W
are sent pre-transposed in bf16 so the kernel needs no on-chip transposes
or casts for them; the mask is sent transposed ([key, query]) in bf16.

Per-core structure:
  - setup: q/k/v projections from xT (PE) with relu+bias epilogues on
    ACT (early groups, hiding in ACT's idle head) and DVE (tensor_scalar
    fused add-bias+max). v is computed in natural [key, h] layout directly
    (lhsT = xT chunk), with its free-axis bias pre-seeded into PSUM by a
    rank-1 PE matmul so the epilogue is a plain max(.,0) copy.
  - main loop: 32 macro-steps (MS) of 4 key-chunks x 512 queries:
    4 S-matmuls -> 2 exps -> mask-mult -> 2 Z-presum adds -> 4 AV matmuls
    + 1 Z ones-matmul. Per-MS engine busy ~= ACT 2076ns / DVE 2048ns /
    PE 1917ns / DMA 1456ns. A few MS per group run the mult/presums on
    gpsimd (otherwise idle) with their AV/Z matmuls deferred to the end of
    the group so the in-order PE queue never waits on slow gpsimd ops.
  - epilogue per 512-query group: Z -> query partitions via k=1 transpose
    matmuls, G = U^T Wo in bf16, y = relu(G/Z + bo) via DVE STT + gpsimd
    relu.

Environment note: this walrus build rejects instructions with more than
one semaphore wait; _split_multiwaits rewrites the Tile output to
single-wait form (extra waits move to preceding NoOps, same semantics).
"""
import ml_dtypes
import numpy as np

import concourse.bass as bass
import concourse.mybir as mybir
from concourse.tile import TileContext
from concourse.bass_utils import run_bass_kernel_spmd

B, N, DIN, H, DOUT = 4, 4096, 128, 128, 128
NQ = N // 2  # 2048 query rows per core
CORES = 8

F32 = mybir.dt.float32
BF16 = mybir.dt.bfloat16
AL = mybir.AluOpType
AF = mybir.ActivationFunctionType

_wfix_ctr = [0]


def _split_multiwaits(nc):
    """This walrus build rejects instructions carrying >1 semaphore wait.

    Move all but the last wait of each instruction onto preceding NoOps on
    the same engine (engine streams execute in order, so semantics hold).
    """
    for func in nc.m.functions:
        for block in func.blocks:
            new_insts = []
            changed = False
            for inst in block.instructions:
                si = inst.sync_info
                waits = list(si.on_wait) if si is not None else []
                if len(waits) > 1:
                    for w in waits[:-1]:
                        _wfix_ctr[0] += 1
                        nop = mybir.InstNoOp(
                            name=f"WFIX-{_wfix_ctr[0]}", ins=[], outs=[]
                        )
                        nop.engine = inst.engine
                        nop.sync_info = mybir.SyncInfo(on_wait=[w], on_update=[])
                        new_insts.append(nop)
                    si.on_wait = [waits[-1]]
                    inst.sync_info = si
                    changed = True
                new_insts.append(inst)
            if changed:
                block.instructions = new_insts


def _build_nc(gps_jj=(0, 4), act_relu_groups=4):
    """gps_jj: which macro-steps (jj index) of each group run the
    mask-mult + Z-presum chain on gpsimd instead of DVE (their AV/Z
    matmuls are deferred to group end). act_relu_groups: how many of the
    earliest projection relus run on ACT (they hide in ACT's idle head);
    the rest run on DVE."""
    nc = bass.Bass()

    xTq_d = nc.dram_tensor("xTq", [DIN, NQ], BF16, kind="ExternalInput")
    xTkv_d = nc.dram_tensor("xTkv", [DIN, N], BF16, kind="ExternalInput")
    maskT_d = nc.dram_tensor("maskT", [N, NQ], BF16, kind="ExternalInput")
    wqT_d = nc.dram_tensor("WqT", [DIN, H], BF16, kind="ExternalInput")
    wkT_d = nc.dram_tensor("WkT", [DIN, H], BF16, kind="ExternalInput")
    wvT_d = nc.dram_tensor("WvT", [DIN, H], BF16, kind="ExternalInput")
    woT_d = nc.dram_tensor("WoT", [H, DOUT], BF16, kind="ExternalInput")
    bq_d = nc.dram_tensor("bq", [H], F32, kind="ExternalInput")
    bk_d = nc.dram_tensor("bk", [H], F32, kind="ExternalInput")
    bv4_d = nc.dram_tensor("bv4", [4 * H], BF16, kind="ExternalInput")
    bo_d = nc.dram_tensor("bo", [DOUT], F32, kind="ExternalInput")
    y_d = nc.dram_tensor("y", [NQ, DOUT], F32, kind="ExternalOutput")

    with TileContext(nc) as tc:
        with tc.tile_pool(name="singles", bufs=1) as singles:
            ones16 = singles.tile([128, 1], BF16, tag="ones16")
            nc.vector.memset(ones16, 1.0)
            onesrow = singles.tile([1, 128], BF16, tag="onesrow")
            nc.vector.memset(onesrow, 1.0)
            one_f32 = singles.tile([1, 1], F32, tag="one_f32")
            nc.vector.memset(one_f32, 1.0)

            bq_sb = singles.tile([128, 1], F32, tag="bq")
            bk_sb = singles.tile([128, 1], F32, tag="bk")
            bo_sb = singles.tile([128, 128], F32, tag="bo")
            bv4_sb = singles.tile([1, 512], BF16, tag="bv4")
            nc.sync.dma_start(
                out=bq_sb, in_=bq_d[:].rearrange("(p o) -> p o", o=1)
            )
            nc.sync.dma_start(
                out=bk_sb, in_=bk_d[:].rearrange("(p o) -> p o", o=1)
            )
            nc.sync.dma_start(
                out=bo_sb,
                in_=bass.AP(tensor=bo_d, offset=0, ap=[[0, 128], [1, 128]]),
            )
            nc.sync.dma_start(
                out=bv4_sb, in_=bv4_d[:].rearrange("(o n) -> o n", o=1)
            )

            wqT = singles.tile([128, 128], BF16, tag="wqT")
            wkT = singles.tile([128, 128], BF16, tag="wkT")
            wvT = singles.tile([128, 128], BF16, tag="wvT")
            woT = singles.tile([128, 128], BF16, tag="woT")
            for w_dram, dst in (
                (wqT_d, wqT), (wkT_d, wkT), (wvT_d, wvT), (woT_d, woT)
            ):
                nc.sync.dma_start(out=dst, in_=w_dram[:])

            xTq = singles.tile([128, 4, 512], BF16, tag="xTq")
            xTkv = singles.tile([128, 8, 512], BF16, tag="xTkv")

            qT = [
                singles.tile([128, 512], BF16, tag=f"qT{g}", name=f"qT{g}")
                for g in range(4)
            ]
            kT = [
                singles.tile([128, 512], BF16, tag=f"kT{g}", name=f"kT{g}")
                for g in range(8)
            ]
            vn = [
                singles.tile([128, 4, 128], BF16, tag=f"vn{g}", name=f"vn{g}")
                for g in range(8)
            ]

            with tc.tile_pool(name="setup_ps", bufs=3, space="PSUM") as sps:
                # Interleave projections so group-0 dependencies clear first.
                # Each step: one x-slice DMA already issued; matmul -> relu.
                def q_proj(g, on_act):
                    nc.sync.dma_start(
                        out=xTq[:, g, :], in_=xTq_d[:, g * 512:(g + 1) * 512]
                    )
                    pq = sps.tile([128, 512], F32, tag="proj", name=f"pq{g}")
                    nc.tensor.matmul(
                        out=pq, lhsT=wqT, rhs=xTq[:, g, :],
                        start=True, stop=True,
                    )
                    if on_act:
                        nc.scalar.activation(
                            out=qT[g][:], in_=pq, func=AF.Relu, bias=bq_sb
                        )
                    else:
                        nc.vector.tensor_scalar(
                            out=qT[g][:], in0=pq, scalar1=bq_sb, scalar2=0.0,
                            op0=AL.add, op1=AL.max,
                        )

                def k_proj(g, on_act):
                    nc.sync.dma_start(
                        out=xTkv[:, g, :], in_=xTkv_d[:, g * 512:(g + 1) * 512]
                    )
                    pk = sps.tile([128, 512], F32, tag="proj", name=f"pk{g}")
                    nc.tensor.matmul(
                        out=pk, lhsT=wkT, rhs=xTkv[:, g, :],
                        start=True, stop=True,
                    )
                    if on_act:
                        nc.scalar.activation(
                            out=kT[g][:], in_=pk, func=AF.Relu, bias=bk_sb
                        )
                    else:
                        nc.vector.tensor_scalar(
                            out=kT[g][:], in0=pk, scalar1=bk_sb, scalar2=0.0,
                            op0=AL.add, op1=AL.max,
                        )

                def v_proj(g):
                    # v natural [key, h]: per chunk lhsT = xT chunk, with the
                    # free-axis bias pre-seeded into PSUM by a rank-1 matmul.
                    pv = sps.tile([128, 4, 128], F32, tag="proj",
                                  name=f"pv{g}")
                    nc.tensor.matmul(
                        out=pv.rearrange("p a b -> p (a b)"),
                        lhsT=onesrow, rhs=bv4_sb,
                        start=True, stop=False, skip_group_check=True,
                    )
                    for c in range(4):
                        nc.tensor.matmul(
                            out=pv[:, c, :],
                            lhsT=xTkv[:, g, c * 128:(c + 1) * 128],
                            rhs=wvT,
                            start=False, stop=True, skip_group_check=True,
                        )
                    nc.vector.tensor_scalar(
                        out=vn[g][:].rearrange("p a b -> p (a b)"),
                        in0=pv.rearrange("p a b -> p (a b)"),
                        scalar1=0.0, scalar2=None, op0=AL.max,
                    )

                # v_proj(g) consumes xTkv chunks 4g..4g+3 = slices g//2, but
                # k_proj(g) loads slice g; emit v after the k that loads it.
                relu_budget = [act_relu_groups]

                def on_act():
                    relu_budget[0] -= 1
                    return relu_budget[0] >= 0

                q_proj(0, on_act())
                k_proj(0, on_act())
                k_proj(1, on_act())
                v_proj(0)
                q_proj(1, on_act())
                k_proj(2, on_act())
                k_proj(3, on_act())
                v_proj(1)
                q_proj(2, on_act())
                k_proj(4, on_act())
                k_proj(5, on_act())
                v_proj(2)
                q_proj(3, on_act())
                k_proj(6, on_act())
                k_proj(7, on_act())
                v_proj(3)
                for g in range(4, 8):
                    v_proj(g)

            _main(nc, tc, maskT_d, y_d, qT, kT, vn, woT, bo_sb,
                  ones16, one_f32, gps_jj)

    _split_multiwaits(nc)
    return nc


def _main(nc, tc, maskT_d, y_d, qT, kT, vn, woT, bo_sb, ones16, one_f32,
          gps_jj):
    with (
        tc.tile_pool(name="mqp", bufs=4) as mqpool,
        tc.tile_pool(name="ep", bufs=6) as epool,
        tc.tile_pool(name="zap", bufs=3) as zapool,
        tc.tile_pool(name="zbp", bufs=6) as zbpool,
        tc.tile_pool(name="tiny", bufs=8) as tinypool,
        tc.tile_pool(name="utsb", bufs=2) as utsbp,
        tc.tile_pool(name="outb", bufs=8) as ypool,
        tc.tile_pool(name="sps2", bufs=2, space="PSUM") as spsum,
        tc.tile_pool(name="utps", bufs=2, space="PSUM") as utpsum,
        tc.tile_pool(name="zps", bufs=1, space="PSUM") as zpsum,
    ):
        for ig in range(4):
            utp = utpsum.tile([128, 512], F32, tag="ut", name=f"utp{ig}")
            zp = zpsum.tile([1, 512], F32, tag="z", name=f"zp{ig}")
            deferred = []  # (E, zb) for gps macro-steps
            n_inline = 8 - len(gps_jj)
            inline_seen = [0]
            defer_av_done = [0]

            def do_av_z(E, zb, jj, first, last):
                for w in range(4):
                    g4, c4 = divmod(4 * jj + w, 4)
                    nc.tensor.matmul(
                        out=utp,
                        lhsT=vn[g4][:, c4, :],
                        rhs=E[:, w * 512:(w + 1) * 512],
                        start=(first and w == 0),
                        stop=(last and w == 3),
                    )
                nc.tensor.matmul(
                    out=zp, lhsT=ones16, rhs=zb,
                    start=first, stop=last,
                )

            for jj in range(8):
                on_gps = jj in gps_jj
                mq = mqpool.tile([128, 4, 512], BF16, tag="mq",
                                 name=f"mq{ig}_{jj}")
                nc.sync.dma_start(
                    out=mq,
                    in_=maskT_d[
                        jj * 512:(jj + 1) * 512,
                        ig * 512:(ig + 1) * 512,
                    ].rearrange("(c p) i -> p c i", p=128),
                )
                E = epool.tile([128, 2048], BF16, tag="E",
                               name=f"E{ig}_{jj}")
                for half in range(2):
                    sp = spsum.tile([128, 2, 512], F32, tag="s",
                                    name=f"sp{ig}_{jj}_{half}")
                    for w in range(2):
                        jc = 4 * jj + 2 * half + w
                        nc.tensor.matmul(
                            out=sp[:, w, :],
                            lhsT=kT[jc // 4][:, (jc % 4) * 128:(jc % 4 + 1) * 128],
                            rhs=qT[ig][:],
                            start=True, stop=True,
                        )
                    nc.scalar.activation(
                        out=E[:, half * 1024:(half + 1) * 1024],
                        in_=sp.rearrange("p a b -> p (a b)"),
                        func=AF.Exp,
                    )
                eng = nc.gpsimd if on_gps else nc.vector
                # P^T = maskT * exp(S^T), all-bf16 (2x on DVE)
                eng.tensor_tensor(
                    out=E[:], in0=E[:],
                    in1=mq[:].rearrange("p a b -> p (a b)"), op=AL.mult,
                )
                # Z partial sums: 4 chunks -> 1 [128, 512] tile
                za = zapool.tile([128, 1024], BF16, tag="za",
                                 name=f"za{ig}_{jj}")
                eng.tensor_tensor(
                    out=za, in0=E[:, 0:1024], in1=E[:, 1024:2048], op=AL.add
                )
                zb = zbpool.tile([128, 512], BF16, tag="zb",
                                 name=f"zb{ig}_{jj}")
                eng.tensor_tensor(
                    out=zb, in0=za[:, 0:512], in1=za[:, 512:1024], op=AL.add
                )
                if on_gps:
                    deferred.append((E, zb, jj))
                else:
                    first = inline_seen[0] == 0
                    inline_seen[0] += 1
                    last = not gps_jj and inline_seen[0] == n_inline
                    do_av_z(E, zb, jj, first, last)
            for i, (E, zb, jj) in enumerate(deferred):
                do_av_z(E, zb, jj, False, i == len(deferred) - 1)

            # --- group epilogue ---
            zrow = tinypool.tile([1, 512], F32, tag="zrow", name=f"zr{ig}")
            nc.vector.tensor_copy(zrow, zp)
            uts = utsbp.tile([128, 512], BF16, tag="uts", name=f"uts{ig}")
            nc.vector.tensor_copy(uts, utp)
            ztp = zpsum.tile([128, 4], F32, tag="epi", name=f"ztp{ig}",
                             padded_shape=[128, 128])
            for i4 in range(4):
                nc.tensor.matmul(
                    out=ztp[:, i4:i4 + 1],
                    lhsT=zrow[:, i4 * 128:(i4 + 1) * 128],
                    rhs=one_f32,
                    start=True, stop=True,
                )
            rz4 = tinypool.tile([128, 4], F32, tag="rz4", name=f"rz4_{ig}")
            nc.vector.reciprocal(rz4, ztp)
            yt = ypool.tile([128, 4, 128], F32, tag="y", name=f"y{ig}")
            for ib in range(4):
                g = zpsum.tile([128, 128], F32, tag="epi",
                               name=f"g{ig}_{ib}")
                nc.tensor.matmul(
                    out=g, lhsT=uts[:, ib * 128:(ib + 1) * 128], rhs=woT,
                    start=True, stop=True,
                )
                u = ypool.tile([128, 128], F32, tag="u", name=f"u{ig}_{ib}")
                nc.vector.scalar_tensor_tensor(
                    out=u, in0=g, scalar=rz4[:, ib:ib + 1],
                    op0=AL.mult, in1=bo_sb, op1=AL.add,
                )
                nc.gpsimd.tensor_scalar(
                    out=yt[:, ib, :], in0=u, scalar1=0.0, scalar2=None,
                    op0=AL.max,
                )
            nc.sync.dma_start(
                out=y_d[ig * 512:(ig + 1) * 512, :].rearrange(
                    "(c p) o -> p c o", p=128
                ),
                in_=yt,
            )


_NC_CACHE = {}


def _get_nc(gps_jj=(0, 4), act_relu_groups=4):
    key = (tuple(gps_jj), act_relu_groups)
    if key not in _NC_CACHE:
        _NC_CACHE[key] = _build_nc(tuple(gps_jj), act_relu_groups)
    return _NC_CACHE[key]


def kernel(x, mask, Wv, bv, Wk, bk, Wq, bq, Wo, bo, _trace=False,
           _gps_jj=(0, 4), _act_relu_groups=4, **_ignored):
    x = np.asarray(x, dtype=np.float32)
    mask = np.asarray(mask, dtype=np.float32)
    bf = ml_dtypes.bfloat16
    weights = {
        "WqT": np.ascontiguousarray(np.asarray(Wq, np.float32).T.astype(bf)),
        "WkT": np.ascontiguousarray(np.asarray(Wk, np.float32).T.astype(bf)),
        "WvT": np.ascontiguousarray(np.asarray(Wv, np.float32).T.astype(bf)),
        "WoT": np.ascontiguousarray(np.asarray(Wo, np.float32).T.astype(bf)),
        "bq": np.ascontiguousarray(np.asarray(bq, np.float32)),
        "bk": np.ascontiguousarray(np.asarray(bk, np.float32)),
        "bv4": np.ascontiguousarray(
            np.tile(np.asarray(bv, np.float32), 4).astype(bf)
        ),
        "bo": np.ascontiguousarray(np.asarray(bo, np.float32)),
    }

    nc = _get_nc(_gps_jj, _act_relu_groups)
    in_maps = []
    for c in range(CORES):
        b, half = divmod(c, 2)
        xb = x[b]
        in_maps.append(
            dict(
                weights,
                xTq=np.ascontiguousarray(
                    xb[half * NQ:(half + 1) * NQ].T.astype(bf)
                ),
                xTkv=np.ascontiguousarray(xb.T.astype(bf)),
                maskT=np.ascontiguousarray(
                    mask[b, half * NQ:(half + 1) * NQ].T.astype(bf)
                ),
            )
        )
    res = run_bass_kernel_spmd(
        nc, in_maps, core_ids=list(range(CORES)), trace=_trace
    )
    out = np.empty((B, N, DOUT), dtype=np.float32)
    for c in range(CORES):
        b, half = divmod(c, 2)
        out[b, half * NQ:(half + 1) * NQ] = res.results[c]["y"]
    if _trace:
        return out, res
    return out
